# revision 2
# baseline (speedup 1.0000x reference)
"""DCP one-to-one matching kernel for Trainium2 (8 NeuronCores).

Data-parallel over the batch dim: 16 batch items, 2 per core.

Per core, per batch item (src_emb/tgt_emb [512, 2048]):
  - PE: logits stripe [128 s, 2048 t] = src_emb_chunk.T @ tgt_emb_chunk (fp32,
    K accumulated over 4x128 chunks in PSUM).
  - ACT: exp(logits / sqrt(512)) -> SBUF, with accum_out giving row sums.
    (No row-max subtraction: |logits| <= ~7 for these inputs, exp is safe in
    fp32 and softmax is mathematically identical.)
  - DVE: reciprocal of row sums; per-row top-8 values + indices (max/max_index);
    top-8 scaled by 1/rowsum = true softmax scores.
  - PE: colsum[t] += r_s * exp[s, t] via matmul with r as stationary operand
    (accumulated across all 16 stripes in PSUM).

Host post-processing (tiny, O(B*(N+S^3))):
  - Replay the greedy one-to-one argmax matching on the per-row top-8
    candidate lists (only 15 rows/cols ever get suppressed, so a row's top-8
    can never be exhausted).
  - Gather matched points, 3x3 cross-covariance, SVD -> R (with reflection
    fix), t = -R @ src_mean + (tgt^T @ colsum) / N.
"""

import math
import sys

import numpy as np

if "/opt/trn_rl_repo" not in sys.path:
    sys.path.insert(0, "/opt/trn_rl_repo")

B, D, N = 16, 512, 2048
NB = 2  # batch items per core
NCORES = 8
P = 128  # partitions
SBLK = N // P  # 16 s-stripes per batch item
KCH = D // P  # 4 contraction chunks
TPAN = 4  # 512-wide t panels
TK = 8  # top-k per row kept on device
NS = 15  # number of greedy matches

_CACHE = {}


def _build_program():
    import concourse.bacc as bacc
    import concourse.tile as tile
    from concourse import mybir

    f32 = mybir.dt.float32
    u32 = mybir.dt.uint32

    nc = bacc.Bacc()
    se = nc.dram_tensor("se", [NB, D, N], f32, kind="ExternalInput")
    te = nc.dram_tensor("te", [NB, D, N], f32, kind="ExternalInput")
    out_vals = nc.dram_tensor("vals", [NB, N, TK], f32, kind="ExternalOutput")
    out_idx = nc.dram_tensor("idx", [NB, N, TK], u32, kind="ExternalOutput")
    out_cs = nc.dram_tensor("cs", [NB, N], f32, kind="ExternalOutput")

    scale = 1.0 / math.sqrt(D)

    with tile.TileContext(nc) as tc:
        with (
            tc.tile_pool(name="emb", bufs=2) as emb_pool,
            tc.tile_pool(name="work", bufs=3) as work_pool,
            tc.tile_pool(name="acc", bufs=2) as acc_pool,
            tc.tile_pool(name="small", bufs=4) as small_pool,
            tc.tile_pool(name="ps_logits", bufs=1, space="PSUM") as psl,
            tc.tile_pool(name="ps_cs", bufs=1, space="PSUM") as psc,
        ):
            for b in range(NB):
                a_sb = emb_pool.tile([P, KCH, N], f32, tag="a")
                b_sb = emb_pool.tile([P, KCH, N], f32, tag="b")
                for k in range(KCH):
                    nc.sync.dma_start(out=a_sb[:, k], in_=se[b, P * k : P * (k + 1), :])
                    nc.sync.dma_start(out=b_sb[:, k], in_=te[b, P * k : P * (k + 1), :])

                vals_acc = acc_pool.tile([P, SBLK, TK], f32, tag="vals")
                idx_acc = acc_pool.tile([P, SBLK, TK], u32, tag="idx")
                cs_tiles = [
                    psc.tile([1, 512], f32, name=f"cs{tp}", tag=f"cs{tp}")
                    for tp in range(TPAN)
                ]
                cs_sb = small_pool.tile([1, N], f32, tag="cs_sb")

                for sb in range(SBLK):
                    lg = psl.tile([P, N], f32, tag="logits")
                    for k in range(KCH):
                        lhsT = a_sb[:, k, P * sb : P * (sb + 1)]
                        for tp in range(TPAN):
                            nc.tensor.matmul(
                                lg[:, 512 * tp : 512 * (tp + 1)],
                                lhsT,
                                b_sb[:, k, 512 * tp : 512 * (tp + 1)],
                                start=(k == 0),
                                stop=(k == KCH - 1),
                            )

                    exp_sb = work_pool.tile([P, N], f32, tag="exp")
                    zpart = small_pool.tile([P, TPAN], f32, tag="zpart")
                    for tp in range(TPAN):
                        nc.scalar.activation(
                            exp_sb[:, 512 * tp : 512 * (tp + 1)],
                            lg[:, 512 * tp : 512 * (tp + 1)],
                            mybir.ActivationFunctionType.Exp,
                            scale=scale,
                            accum_out=zpart[:, tp : tp + 1],
                        )

                    z = small_pool.tile([P, 1], f32, tag="z")
                    nc.vector.reduce_sum(z, zpart, axis=mybir.AxisListType.X)
                    r = small_pool.tile([P, 1], f32, tag="r")
                    nc.vector.reciprocal(r, z)

                    v8 = small_pool.tile([P, TK], f32, tag="v8")
                    nc.vector.max(out=v8, in_=exp_sb)
                    nc.vector.max_index(out=idx_acc[:, sb], in_max=v8, in_values=exp_sb)
                    nc.vector.tensor_scalar_mul(vals_acc[:, sb], v8, r)

                    for tp in range(TPAN):
                        nc.tensor.matmul(
                            cs_tiles[tp][:, :],
                            r,
                            exp_sb[:, 512 * tp : 512 * (tp + 1)],
                            start=(sb == 0),
                            stop=(sb == SBLK - 1),
                        )

                for tp in range(TPAN):
                    nc.scalar.copy(cs_sb[:, 512 * tp : 512 * (tp + 1)], cs_tiles[tp][:, :])

                nc.sync.dma_start(
                    out=out_vals[b].rearrange("(sb p) k -> p sb k", p=P),
                    in_=vals_acc,
                )
                nc.sync.dma_start(
                    out=out_idx[b].rearrange("(sb p) k -> p sb k", p=P),
                    in_=idx_acc,
                )
                nc.sync.dma_start(out=out_cs[b : b + 1, :], in_=cs_sb)

    nc.finalize()
    return nc


def _run_device(src_embedding, tgt_embedding, trace=False):
    from concourse.bass_utils import run_bass_kernel_spmd

    if "nc" not in _CACHE:
        _CACHE["nc"] = _build_program()
    nc = _CACHE["nc"]

    in_maps = []
    for c in range(NCORES):
        in_maps.append(
            {
                "se": np.ascontiguousarray(src_embedding[NB * c : NB * (c + 1)]),
                "te": np.ascontiguousarray(tgt_embedding[NB * c : NB * (c + 1)]),
            }
        )
    res = run_bass_kernel_spmd(
        nc, in_maps, core_ids=list(range(NCORES)), trace=trace
    )
    return res


def _greedy_match(vals, idxs):
    """Replay reference sample_match on per-row top-8 candidates.

    vals/idxs: [N, TK] descending per row. Returns (rows, cols) each [NS].
    """
    n = vals.shape[0]
    flat_vals = vals.reshape(-1)
    flat_rows = np.repeat(np.arange(n, dtype=np.int64), TK)
    flat_cols = idxs.reshape(-1).astype(np.int64)
    # Reference picks the first (row-major) occurrence of the max, so order
    # by value desc, then (row, col) asc.
    order = np.lexsort((flat_cols, flat_rows, -flat_vals.astype(np.float64)))
    row_used = np.zeros(n, dtype=bool)
    col_used = np.zeros(n, dtype=bool)
    rows = np.empty(NS, dtype=np.int32)
    cols = np.empty(NS, dtype=np.int32)
    k = 0
    for i in order:
        r = flat_rows[i]
        c = flat_cols[i]
        if row_used[r] or col_used[c]:
            continue
        rows[k] = r
        cols[k] = c
        row_used[r] = True
        col_used[c] = True
        k += 1
        if k == NS:
            break
    assert k == NS, "candidate list exhausted before 15 matches"
    return rows, cols


def _finish_host(src, tgt, vals, idxs, colsum):
    """src/tgt: [N, 3] f32; vals/idxs: [N, TK]; colsum: [N]. Returns R, t."""
    rows, cols = _greedy_match(vals, idxs)
    topk_src = src[rows].T.astype(np.float32)  # [3, NS]
    topk_tgt = tgt[cols].T.astype(np.float32)

    src_c = topk_src - topk_src.mean(axis=1, keepdims=True)
    tgt_c = topk_tgt - topk_tgt.mean(axis=1, keepdims=True)
    H = (src_c @ tgt_c.T).astype(np.float32)

    u, s, vh = np.linalg.svd(H)
    v = vh.T
    ut = u.T
    r = v @ ut
    det = np.linalg.det(r)
    reflect = np.array([1.0, 1.0, -1.0], dtype=np.float32)
    r_fix = (v * reflect[None, :]) @ ut
    R = r_fix if det < 0 else r

    src_mean = src.mean(axis=0)  # [3]
    src_corr_mean = (colsum @ tgt) / np.float32(N)  # [3]
    t = (-R) @ src_mean + src_corr_mean
    return R.astype(np.float32), t.astype(np.float32)


def kernel(src_embedding, tgt_embedding, src, tgt):
    src_embedding = np.asarray(src_embedding, dtype=np.float32)
    tgt_embedding = np.asarray(tgt_embedding, dtype=np.float32)
    src = np.asarray(src, dtype=np.float32)
    tgt = np.asarray(tgt, dtype=np.float32)

    res = _run_device(src_embedding, tgt_embedding)

    R = np.empty((B, 3, 3), dtype=np.float32)
    t = np.empty((B, 3), dtype=np.float32)
    for b in range(B):
        core, slot = divmod(b, NB)
        out = res.results[core]
        R[b], t[b] = _finish_host(
            src[b], tgt[b], out["vals"][slot], out["idx"][slot], out["cs"][slot]
        )
    return R, t


# revision 8
# speedup vs baseline: 2.9312x; 2.9312x over previous
"""DCP one-to-one matching kernel for Trainium2 (8 NeuronCores).

Data-parallel over the batch dim: 16 batch items, 2 per core.

Per core, per batch item (src_emb/tgt_emb [512, 2048]):
  - PE: logits stripe [128 s, 2048 t] = src_emb_chunk.T @ tgt_emb_chunk using
    float32r matmuls (single-pass fp32, ~TF32-level precision) accumulated
    over 4x128 K chunks in PSUM.
  - ACT: exp(logits / sqrt(512)) -> SBUF, with accum_out giving row sums.
    (No row-max subtraction: |logits| <= ~7 for these inputs, exp is safe in
    fp32 and softmax is mathematically identical.)
  - DVE: reciprocal of row sums; per-row top-8 values + indices (max/max_index);
    top-8 scaled by 1/rowsum = softmax scores.
  - PE: colsum[t] += r_s * exp[s, t] via matmul with r as the stationary
    operand (accumulated across all 16 stripes in PSUM). These matmuls are
    emitted one stripe late so they never stall the PE behind the softmax
    chain.

Host post-processing (tiny):
  - f32r logits carry ~1e-3 absolute noise, which is too coarse to decide
    argmax order among the top candidates. The top ~256 candidates per batch
    are therefore re-scored exactly on the host (f64 dot of the two 512-dim
    embedding columns + the device row-normalizer), then the greedy
    one-to-one matching is replayed on the candidate list. Only 15 rows/cols
    ever get suppressed, so a row's top-8 can never be exhausted.
  - Gather matched points, 3x3 cross-covariance, SVD -> R (with reflection
    fix), t = -R @ src_mean + (tgt^T @ colsum) / N.
"""

import math
import sys

import numpy as np

if "/opt/trn_rl_repo" not in sys.path:
    sys.path.insert(0, "/opt/trn_rl_repo")

B, D, N = 16, 512, 2048
NB = 2  # batch items per core
NCORES = 8
P = 128  # partitions
SBLK = N // P  # 16 s-stripes per batch item
KCH = D // P  # 4 contraction chunks
TPAN = 4  # 512-wide t panels
TK = 8  # top-k per row kept on device
NS = 15  # number of greedy matches
TOPM = 256  # candidates re-scored exactly on host per batch

_CACHE = {}


def _build_program():
    import concourse.bacc as bacc
    import concourse.tile as tile
    from concourse import mybir

    f32 = mybir.dt.float32
    f32r = mybir.dt.float32r
    u32 = mybir.dt.uint32

    nc = bacc.Bacc()
    # declared f32r (same 4-byte layout as f32) so the PE runs single-pass
    # reduced-precision fp32 matmuls; host rescoring absorbs the noise
    se = nc.dram_tensor("se", [NB, D, N], f32r, kind="ExternalInput")
    te = nc.dram_tensor("te", [NB, D, N], f32r, kind="ExternalInput")
    out_vals = nc.dram_tensor("vals", [NB, N, TK], f32, kind="ExternalOutput")
    out_idx = nc.dram_tensor("idx", [NB, N, TK], u32, kind="ExternalOutput")
    out_cs = nc.dram_tensor("cs", [NB, N], f32, kind="ExternalOutput")
    out_rz = nc.dram_tensor("rz", [NB, N], f32, kind="ExternalOutput")

    scale = 1.0 / math.sqrt(D)

    with tile.TileContext(nc) as tc:
        with (
            tc.tile_pool(name="emb", bufs=2) as emb_pool,
            tc.tile_pool(name="work", bufs=3) as work_pool,
            tc.tile_pool(name="acc", bufs=2) as acc_pool,
            tc.tile_pool(name="small", bufs=6) as small_pool,
            tc.tile_pool(name="ps_logits", bufs=4, space="PSUM") as psl,
            tc.tile_pool(name="ps_cs", bufs=1, space="PSUM") as psc,
        ):
            for b in range(NB):
                a_sb = emb_pool.tile([P, KCH, N], f32r, tag="a")
                b_sb = emb_pool.tile([P, KCH, N], f32r, tag="b")
                for k in range(KCH):
                    nc.sync.dma_start(out=a_sb[:, k], in_=se[b, P * k : P * (k + 1), :])
                    nc.sync.dma_start(out=b_sb[:, k], in_=te[b, P * k : P * (k + 1), :])

                vals_acc = acc_pool.tile([P, SBLK, TK], f32, tag="vals")
                idx_acc = acc_pool.tile([P, SBLK, TK], u32, tag="idx")
                rz_acc = acc_pool.tile([P, SBLK], f32, tag="rz")
                cs_tiles = [
                    psc.tile([1, 512], f32, name=f"cs{tp}", tag=f"cs{tp}")
                    for tp in range(TPAN)
                ]
                cs_sb = small_pool.tile([1, N], f32, tag="cs_sb")

                # pending colsum matmuls, emitted one stripe late so the PE
                # never waits on the softmax chain of the current stripe
                pending_cs = None

                for sb in range(SBLK):
                    panels = []
                    for tp in range(TPAN):
                        lg = psl.tile([P, 512], f32, name=f"lg{sb}_{tp}", tag="logits")
                        for k in range(KCH):
                            nc.tensor.matmul(
                                lg,
                                a_sb[:, k, P * sb : P * (sb + 1)],
                                b_sb[:, k, 512 * tp : 512 * (tp + 1)],
                                start=(k == 0),
                                stop=(k == KCH - 1),
                            )
                        panels.append(lg)

                    if pending_cs is not None:
                        prev_r, prev_exp, prev_sb = pending_cs
                        for tp in range(TPAN):
                            nc.tensor.matmul(
                                cs_tiles[tp][:, :],
                                prev_r,
                                prev_exp[:, 512 * tp : 512 * (tp + 1)],
                                start=(prev_sb == 0),
                                stop=(prev_sb == SBLK - 1),
                            )

                    exp_sb = work_pool.tile([P, N], f32, tag="exp")
                    zpart = small_pool.tile([P, TPAN], f32, tag="zpart")
                    for tp in range(TPAN):
                        nc.scalar.activation(
                            exp_sb[:, 512 * tp : 512 * (tp + 1)],
                            panels[tp],
                            mybir.ActivationFunctionType.Exp,
                            scale=scale,
                            accum_out=zpart[:, tp : tp + 1],
                        )

                    z = small_pool.tile([P, 1], f32, tag="z")
                    nc.vector.reduce_sum(z, zpart, axis=mybir.AxisListType.X)
                    r = small_pool.tile([P, 1], f32, tag="r")
                    nc.vector.reciprocal(r, z)
                    nc.vector.tensor_copy(rz_acc[:, sb : sb + 1], r)

                    v8 = small_pool.tile([P, TK], f32, tag="v8")
                    nc.vector.max(out=v8, in_=exp_sb)
                    nc.vector.max_index(out=idx_acc[:, sb], in_max=v8, in_values=exp_sb)
                    nc.vector.tensor_scalar_mul(vals_acc[:, sb], v8, r)

                    pending_cs = (r, exp_sb, sb)

                # flush the last stripe's colsum matmuls
                prev_r, prev_exp, prev_sb = pending_cs
                for tp in range(TPAN):
                    nc.tensor.matmul(
                        cs_tiles[tp][:, :],
                        prev_r,
                        prev_exp[:, 512 * tp : 512 * (tp + 1)],
                        start=(prev_sb == 0),
                        stop=(prev_sb == SBLK - 1),
                    )

                for tp in range(TPAN):
                    nc.scalar.copy(cs_sb[:, 512 * tp : 512 * (tp + 1)], cs_tiles[tp][:, :])

                nc.sync.dma_start(
                    out=out_vals[b].rearrange("(sb p) k -> p sb k", p=P),
                    in_=vals_acc,
                )
                nc.sync.dma_start(
                    out=out_idx[b].rearrange("(sb p) k -> p sb k", p=P),
                    in_=idx_acc,
                )
                nc.sync.dma_start(
                    out=out_rz[b].rearrange("(sb p) -> p sb", p=P), in_=rz_acc
                )
                nc.sync.dma_start(out=out_cs[b : b + 1, :], in_=cs_sb)

    nc.finalize()
    return nc


def _run_device(src_embedding, tgt_embedding, trace=False):
    from concourse.bass_utils import run_bass_kernel_spmd

    if "nc" not in _CACHE:
        _CACHE["nc"] = _build_program()
    nc = _CACHE["nc"]

    in_maps = []
    for c in range(NCORES):
        in_maps.append(
            {
                "se": np.ascontiguousarray(src_embedding[NB * c : NB * (c + 1)]),
                "te": np.ascontiguousarray(tgt_embedding[NB * c : NB * (c + 1)]),
            }
        )
    res = run_bass_kernel_spmd(
        nc, in_maps, core_ids=list(range(NCORES)), trace=trace
    )
    return res


def _greedy_match(vals, idxs, rz, se_b, te_b):
    """Replay reference sample_match on per-row top-8 candidates.

    vals/idxs: [N, TK] descending per row (f32r-noisy), rz: [N] row 1/Z.
    se_b/te_b: [D, N] f32 embeddings for exact rescoring.
    Returns (rows, cols) each [NS].
    """
    n = vals.shape[0]
    flat_vals = vals.reshape(-1)
    flat_rows = np.repeat(np.arange(n, dtype=np.int64), TK)
    flat_cols = idxs.reshape(-1).astype(np.int64)

    # top-M candidates by approximate score
    m = min(TOPM, flat_vals.size)
    top = np.argpartition(-flat_vals, m - 1)[:m]
    rows_m = flat_rows[top]
    cols_m = flat_cols[top]

    # exact rescore: f64 logits + device row normalizer
    lo = np.einsum(
        "ds,ds->s",
        se_b[:, rows_m].astype(np.float64),
        te_b[:, cols_m].astype(np.float64),
    ) / math.sqrt(D)
    sc = np.exp(lo) * rz[rows_m].astype(np.float64)

    order = np.lexsort((cols_m, rows_m, -sc))
    row_used = np.zeros(n, dtype=bool)
    col_used = np.zeros(n, dtype=bool)
    rows = np.empty(NS, dtype=np.int32)
    cols = np.empty(NS, dtype=np.int32)
    k = 0
    for i in order:
        r = rows_m[i]
        c = cols_m[i]
        if row_used[r] or col_used[c]:
            continue
        rows[k] = r
        cols[k] = c
        row_used[r] = True
        col_used[c] = True
        k += 1
        if k == NS:
            break
    assert k == NS, "candidate list exhausted before 15 matches"
    return rows, cols


def _finish_host(src, tgt, vals, idxs, rz, colsum, se_b, te_b):
    """src/tgt: [N, 3] f32. Returns R, t."""
    rows, cols = _greedy_match(vals, idxs, rz, se_b, te_b)
    topk_src = src[rows].T.astype(np.float32)  # [3, NS]
    topk_tgt = tgt[cols].T.astype(np.float32)

    src_c = topk_src - topk_src.mean(axis=1, keepdims=True)
    tgt_c = topk_tgt - topk_tgt.mean(axis=1, keepdims=True)
    H = (src_c @ tgt_c.T).astype(np.float32)

    u, s, vh = np.linalg.svd(H)
    v = vh.T
    ut = u.T
    r = v @ ut
    det = np.linalg.det(r)
    reflect = np.array([1.0, 1.0, -1.0], dtype=np.float32)
    r_fix = (v * reflect[None, :]) @ ut
    R = r_fix if det < 0 else r

    src_mean = src.mean(axis=0)  # [3]
    src_corr_mean = (colsum @ tgt) / np.float32(N)  # [3]
    t = (-R) @ src_mean + src_corr_mean
    return R.astype(np.float32), t.astype(np.float32)


def kernel(src_embedding, tgt_embedding, src, tgt):
    src_embedding = np.asarray(src_embedding, dtype=np.float32)
    tgt_embedding = np.asarray(tgt_embedding, dtype=np.float32)
    src = np.asarray(src, dtype=np.float32)
    tgt = np.asarray(tgt, dtype=np.float32)

    res = _run_device(src_embedding, tgt_embedding)

    R = np.empty((B, 3, 3), dtype=np.float32)
    t = np.empty((B, 3), dtype=np.float32)
    for b in range(B):
        core, slot = divmod(b, NB)
        out = res.results[core]
        R[b], t[b] = _finish_host(
            src[b],
            tgt[b],
            out["vals"][slot],
            out["idx"][slot],
            out["rz"][slot],
            out["cs"][slot],
            src_embedding[b],
            tgt_embedding[b],
        )
    return R, t


# revision 9
# speedup vs baseline: 4.2396x; 1.4464x over previous
"""DCP one-to-one matching kernel for Trainium2 (8 NeuronCores).

Data-parallel over the batch dim: 16 batch items, 2 per core.

Per core, per batch item (src_emb/tgt_emb [512, 2048]):
  - PE: logits stripe [128 s, 2048 t] = src_emb_chunk.T @ tgt_emb_chunk in
    bf16 (fp32 PSUM accumulate), K accumulated over 4x128 chunks.
  - ACT: exp(logits / sqrt(512)) -> SBUF fp32, with accum_out row sums.
    (No row-max subtraction: |logits| <= ~7 for these inputs, exp is safe in
    fp32 and softmax is mathematically identical.)
  - DVE: reciprocal of row sums; per-row top-8 values + indices (max/max_index
    on the fp32 exp, so quantization can't create duplicate values);
    top-8 scaled by 1/rowsum = approximate softmax scores.
  - PE: colsum[t] += r_s * exp[s, t] via fp32 matmuls with r as the stationary
    operand, 4 t-panels run concurrently in distinct PE column groups
    (tile_position), accumulated across all 16 stripes in one PSUM bank.
    Emitted one stripe late so the PE never stalls behind the softmax chain.

Host post-processing (small):
  - bf16 logits carry ~1% relative noise: fine for *selecting* the top ~256
    candidates per batch, not for ordering them. The candidates (and their
    rows' softmax normalizers Z) are re-scored exactly from the fp32
    embeddings on the host (one [rows x 512] @ [512 x 2048] sgemm per batch),
    then the greedy one-to-one matching is replayed on the candidate list.
    Only 15 rows/cols ever get suppressed, so a row's top-8 can never be
    exhausted.
  - Gather matched points, 3x3 cross-covariance, SVD -> R (with reflection
    fix), t = -R @ src_mean + (tgt^T @ colsum) / N.
"""

import math
import sys

import numpy as np

if "/opt/trn_rl_repo" not in sys.path:
    sys.path.insert(0, "/opt/trn_rl_repo")

B, D, N = 16, 512, 2048
NB = 2  # batch items per core
NCORES = 8
P = 128  # partitions
SBLK = N // P  # 16 s-stripes per batch item
KCH = D // P  # 4 contraction chunks
TPAN = 4  # 512-wide t panels
TK = 8  # top-k per row kept on device
NS = 15  # number of greedy matches
TOPM = 256  # candidates re-scored exactly on host per batch

_CACHE = {}


def _build_program():
    import concourse.bacc as bacc
    import concourse.tile as tile
    from concourse import mybir

    f32 = mybir.dt.float32
    bf16 = mybir.dt.bfloat16
    u32 = mybir.dt.uint32

    nc = bacc.Bacc()
    se = nc.dram_tensor("se", [NB, D, N], bf16, kind="ExternalInput")
    te = nc.dram_tensor("te", [NB, D, N], bf16, kind="ExternalInput")
    out_vals = nc.dram_tensor("vals", [NB, N, TK], f32, kind="ExternalOutput")
    out_idx = nc.dram_tensor("idx", [NB, N, TK], u32, kind="ExternalOutput")
    out_cs = nc.dram_tensor("cs", [NB, N], f32, kind="ExternalOutput")

    scale = 1.0 / math.sqrt(D)

    with tile.TileContext(nc) as tc:
        with (
            tc.tile_pool(name="emb", bufs=2) as emb_pool,
            tc.tile_pool(name="work", bufs=3) as work_pool,
            tc.tile_pool(name="acc", bufs=2) as acc_pool,
            tc.tile_pool(name="small", bufs=6) as small_pool,
            tc.tile_pool(name="ps_logits", bufs=7, space="PSUM") as psl,
            tc.tile_pool(name="ps_cs", bufs=1, space="PSUM") as psc,
        ):
            for b in range(NB):
                a_sb = emb_pool.tile([P, KCH, N], bf16, tag="a")
                b_sb = emb_pool.tile([P, KCH, N], bf16, tag="b")
                for k in range(KCH):
                    nc.sync.dma_start(out=a_sb[:, k], in_=se[b, P * k : P * (k + 1), :])
                    nc.sync.dma_start(out=b_sb[:, k], in_=te[b, P * k : P * (k + 1), :])

                vals_acc = acc_pool.tile([P, SBLK, TK], f32, tag="vals")
                idx_acc = acc_pool.tile([P, SBLK, TK], u32, tag="idx")
                # one PSUM bank; t-panel tp accumulates in partition row 32*tp
                cs_all = psc.tile([P, 512], f32, tag="cs")
                cs_sb = small_pool.tile([P, 512], f32, tag="cs_sb")

                # pending colsum matmuls, emitted one stripe late so the PE
                # never waits on the softmax chain of the current stripe
                pending_cs = None

                def emit_cs(pend):
                    prev_r, prev_exp, prev_sb = pend
                    for tp in range(TPAN):
                        nc.tensor.matmul(
                            cs_all[32 * tp : 32 * tp + 1, :],
                            prev_r,
                            prev_exp[:, 512 * tp : 512 * (tp + 1)],
                            start=(prev_sb == 0),
                            stop=(prev_sb == SBLK - 1),
                            tile_position=(0, 32 * tp),
                        )

                for sb in range(SBLK):
                    panels = []
                    for tp in range(TPAN):
                        lg = psl.tile([P, 512], f32, name=f"lg{sb}_{tp}", tag="logits")
                        for k in range(KCH):
                            nc.tensor.matmul(
                                lg,
                                a_sb[:, k, P * sb : P * (sb + 1)],
                                b_sb[:, k, 512 * tp : 512 * (tp + 1)],
                                start=(k == 0),
                                stop=(k == KCH - 1),
                            )
                        panels.append(lg)

                    if pending_cs is not None:
                        emit_cs(pending_cs)

                    exp_sb = work_pool.tile([P, N], f32, tag="exp")
                    zpart = small_pool.tile([P, TPAN], f32, tag="zpart")
                    for tp in range(TPAN):
                        nc.scalar.activation(
                            exp_sb[:, 512 * tp : 512 * (tp + 1)],
                            panels[tp],
                            mybir.ActivationFunctionType.Exp,
                            scale=scale,
                            accum_out=zpart[:, tp : tp + 1],
                        )

                    z = small_pool.tile([P, 1], f32, tag="z")
                    nc.vector.reduce_sum(z, zpart, axis=mybir.AxisListType.X)
                    r = small_pool.tile([P, 1], f32, tag="r")
                    nc.vector.reciprocal(r, z)

                    v8 = small_pool.tile([P, TK], f32, tag="v8")
                    nc.vector.max(out=v8, in_=exp_sb)
                    nc.vector.max_index(out=idx_acc[:, sb], in_max=v8, in_values=exp_sb)
                    nc.vector.tensor_scalar_mul(vals_acc[:, sb], v8, r)

                    pending_cs = (r, exp_sb, sb)

                emit_cs(pending_cs)

                for tp in range(TPAN):
                    nc.scalar.copy(
                        cs_sb[32 * tp : 32 * tp + 1, :],
                        cs_all[32 * tp : 32 * tp + 1, :],
                    )
                    nc.sync.dma_start(
                        out=out_cs[b : b + 1, 512 * tp : 512 * (tp + 1)],
                        in_=cs_sb[32 * tp : 32 * tp + 1, :],
                    )

                nc.sync.dma_start(
                    out=out_vals[b].rearrange("(sb p) k -> p sb k", p=P),
                    in_=vals_acc,
                )
                nc.sync.dma_start(
                    out=out_idx[b].rearrange("(sb p) k -> p sb k", p=P),
                    in_=idx_acc,
                )

    nc.finalize()
    return nc


def _run_device(se_bf, te_bf, trace=False):
    from concourse.bass_utils import run_bass_kernel_spmd

    if "nc" not in _CACHE:
        _CACHE["nc"] = _build_program()
    nc = _CACHE["nc"]

    in_maps = []
    for c in range(NCORES):
        in_maps.append(
            {
                "se": np.ascontiguousarray(se_bf[NB * c : NB * (c + 1)]),
                "te": np.ascontiguousarray(te_bf[NB * c : NB * (c + 1)]),
            }
        )
    res = run_bass_kernel_spmd(
        nc, in_maps, core_ids=list(range(NCORES)), trace=trace
    )
    return res


def _greedy_match(vals, idxs, se_b, te_b):
    """Replay reference sample_match on per-row top-8 candidates.

    vals/idxs: [N, TK] descending per row (bf16-noisy approximate scores).
    se_b/te_b: [D, N] f32 embeddings for exact rescoring.
    Returns (rows, cols) each [NS].
    """
    n = vals.shape[0]
    flat_vals = vals.reshape(-1)
    flat_rows = np.repeat(np.arange(n, dtype=np.int64), TK)
    flat_cols = idxs.reshape(-1).astype(np.int64)

    # top-M candidates by approximate score
    m = min(TOPM, flat_vals.size)
    top = np.argpartition(-flat_vals, m - 1)[:m]
    rows_m = flat_rows[top]
    cols_m = flat_cols[top]

    # exact rescore: fp32 logits for every candidate row (also yields the
    # exact softmax normalizer Z), then f64 softmax values
    urows, inv = np.unique(rows_m, return_inverse=True)
    logits = (se_b[:, urows].T @ te_b) * np.float32(1.0 / math.sqrt(D))  # [R, N]
    le = np.exp(logits.astype(np.float64))
    zrow = le.sum(axis=1)  # [R]
    sc = le[inv, cols_m] / zrow[inv]

    order = np.lexsort((cols_m, rows_m, -sc))
    row_used = np.zeros(n, dtype=bool)
    col_used = np.zeros(n, dtype=bool)
    rows = np.empty(NS, dtype=np.int32)
    cols = np.empty(NS, dtype=np.int32)
    k = 0
    for i in order:
        r = rows_m[i]
        c = cols_m[i]
        if row_used[r] or col_used[c]:
            continue
        rows[k] = r
        cols[k] = c
        row_used[r] = True
        col_used[c] = True
        k += 1
        if k == NS:
            break
    assert k == NS, "candidate list exhausted before 15 matches"
    return rows, cols


def _finish_host(src, tgt, vals, idxs, colsum, se_b, te_b):
    """src/tgt: [N, 3] f32. Returns R, t."""
    rows, cols = _greedy_match(vals, idxs, se_b, te_b)
    topk_src = src[rows].T.astype(np.float32)  # [3, NS]
    topk_tgt = tgt[cols].T.astype(np.float32)

    src_c = topk_src - topk_src.mean(axis=1, keepdims=True)
    tgt_c = topk_tgt - topk_tgt.mean(axis=1, keepdims=True)
    H = (src_c @ tgt_c.T).astype(np.float32)

    u, s, vh = np.linalg.svd(H)
    v = vh.T
    ut = u.T
    r = v @ ut
    det = np.linalg.det(r)
    reflect = np.array([1.0, 1.0, -1.0], dtype=np.float32)
    r_fix = (v * reflect[None, :]) @ ut
    R = r_fix if det < 0 else r

    src_mean = src.mean(axis=0)  # [3]
    src_corr_mean = (colsum @ tgt) / np.float32(N)  # [3]
    t = (-R) @ src_mean + src_corr_mean
    return R.astype(np.float32), t.astype(np.float32)


def kernel(src_embedding, tgt_embedding, src, tgt):
    import ml_dtypes

    src_embedding = np.asarray(src_embedding, dtype=np.float32)
    tgt_embedding = np.asarray(tgt_embedding, dtype=np.float32)
    src = np.asarray(src, dtype=np.float32)
    tgt = np.asarray(tgt, dtype=np.float32)

    se_bf = src_embedding.astype(ml_dtypes.bfloat16)
    te_bf = tgt_embedding.astype(ml_dtypes.bfloat16)
    res = _run_device(se_bf, te_bf)

    R = np.empty((B, 3, 3), dtype=np.float32)
    t = np.empty((B, 3), dtype=np.float32)
    for b in range(B):
        core, slot = divmod(b, NB)
        out = res.results[core]
        R[b], t[b] = _finish_host(
            src[b],
            tgt[b],
            out["vals"][slot],
            out["idx"][slot],
            out["cs"][slot],
            src_embedding[b],
            tgt_embedding[b],
        )
    return R, t
